# revision 19
# baseline (speedup 1.0000x reference)
"""Butterworth 4th-order lowpass (2 cascaded biquads) on 8 TRN2 NeuronCores.

Algorithm: block state-space decomposition of the IIR cascade.
  - Chunk the time axis into L=128 blocks. Within a chunk, the zero-state
    response is a lower-triangular Toeplitz matmul y_zs = H @ x_chunk (PE).
  - Chunk-boundary states follow s_k = M s_{k-1} + f_k with M = A^L.
    Diagonalize M (2 conjugate eigenpairs); each complex mode is solved by
    a first-order REAL scan (DVE tensor_tensor_scan) via the rotation
    trick  m_k = r m_{k-1} + e^{-i th k} g_k,  shat_k = e^{+i th k} m_k.
  - The state correction is a K=4 matmul y += G'' @ S accumulated in PSUM.

Performance structure (~196 us/core vs 234 us baseline; HW-profiled):
  - fp16 everywhere on the PE: fp32/fp32r matmuls double-pump the array
    (fp32_mode=LOW_HIGH, 2x LDWEIGHTS + 2x MATMUL passes) while fp16
    runs single-pass. fp16 over bf16 because the Toeplitz matmul
    amplifies coefficient rounding ~10x (|x|-scale products cancel to
    |y|-scale outputs); fp16's 10 mantissa bits keep end-to-end error
    ~2e-3 against the 2e-2 gate (bf16 measured 1.3e-2). States |shat|
    <= ~8e3 fit fp16 range. The DVE chunk scan stays fp32.
  - x is pre-cast to fp16 on the HOST and y is returned fp16 (cast back
    to f32 on host): halves both HBM streams to 6.1 MB/core each.
  - y is computed CHUNK-MAJOR directly in PSUM (no output transposes):
      y_blk[c, t] = X[c,:] @ H^T  +  S[:,c]^T @ G''^T
    with the DATA (Xt block / state block) as the matmul stationary and
    the constant (H^T / G''^T) as the moving operand, so one PSUM->SBUF
    copy feeds the store DMA. Input transposes stay on the PE (fp16,
    single-pass): DMA-xbar transposes were tried and are individually
    fast but the compiler serializes them against all SBUF<->SBUF DMA
    traffic (deadlock guard), which cost more than they saved.
  - loads ride the SP HWDGE ring; stores ride the ACT HWDGE ring, so
    the two HBM streams drain in parallel. The f-bounce, swaps and
    state gathers go through gpsimd SWDGE.
  - PSUM->SBUF copies alternate scalar/vector (gpsimd cannot touch PSUM
    and its tensor_copy is ~6x slower anyway - measured).
Sharding: 256 independent signals, 32 per core, no cross-core comm.
"""
import numpy as np
from contextlib import ExitStack

import concourse.bass as bass
import concourse.tile as tile
from concourse import bacc, mybir
from concourse.bass_utils import run_bass_kernel_spmd

dt = mybir.dt

B, C, T_FULL = 32, 8, 96000
NPIPE = 2                      # pipeline segments per core
N_CORES = 8
NSIG = (B * C) // N_CORES      # 32 signals per core
L = 120                        # chunk length (96000/120=800; fused H+G needs L+4<=128)


# ---------------------------------------------------------------- host math
def derive_constants(sos: np.ndarray, K: int, HS: int = 16):
    """Constant matrices for the block SSM, float64. K = chunks per signal."""
    sos = sos.astype(np.float64)
    (b0, b1, b2, a1, a2), (B0, B1, B2, A1, A2) = [
        (s[0] / s[3], s[1] / s[3], s[2] / s[3], s[4] / s[3], s[5] / s[3])
        for s in sos
    ]
    c1, c2 = b1 - b0 * a1, b2 - b0 * a2
    A = np.array([
        [-a1, -a2, 0.0, 0.0],
        [1.0, 0.0, 0.0, 0.0],
        [c1, c2, -A1, -A2],
        [0.0, 0.0, 1.0, 0.0],
    ])
    Bv = np.array([1.0, 0.0, b0, 0.0])
    Cv = np.array([B0 * c1, B0 * c2, B1 - B0 * A1, B2 - B0 * A2])
    D = B0 * b0

    h = np.zeros(L)
    h[0] = D
    s = Bv.copy()
    for t in range(1, L):
        h[t] = Cv @ s
        s = A @ s
    H = np.zeros((L, L))
    for j in range(L):
        H[j:, j] = h[: L - j]

    Fm = np.zeros((4, L))
    Ap = np.eye(4)
    for j in range(L - 1, -1, -1):
        Fm[:, j] = Ap @ Bv
        Ap = A @ Ap
    G = np.zeros((L, 4))
    Ap = np.eye(4)
    for t in range(L):
        G[t, :] = Cv @ Ap
        Ap = A @ Ap

    M = np.linalg.matrix_power(A, L)
    lam, V = np.linalg.eig(M)
    idx = [i for i in range(4) if lam[i].imag > 0]
    assert len(idx) == 2, lam
    lam2, V2 = lam[idx], V[:, idx]
    Vinv2 = np.linalg.inv(V)[idx, :]

    Fmod = Vinv2 @ Fm                      # (2, L) complex
    Fp = np.stack([Fmod[0].real, Fmod[0].imag, Fmod[1].real, Fmod[1].imag])
    GV = G @ V2                            # (L, 2) complex
    Gpp = np.stack([2 * GV[:, 0].real, -2 * GV[:, 0].imag,
                    2 * GV[:, 1].real, -2 * GV[:, 1].imag], axis=1)

    r, th = np.abs(lam2), np.angle(lam2)
    k = np.arange(K)
    CCh = np.zeros((4 * HS, K), dtype=np.float64)
    SSh = np.zeros((4 * HS, K), dtype=np.float64)
    for a in range(4):
        e = a // 2
        CCh[a * HS:(a + 1) * HS, :] = np.cos(th[e] * k)[None, :]
        SSh[a * HS:(a + 1) * HS, :] = (1.0 if a % 2 == 0 else -1.0) * \
            np.sin(th[e] * k)[None, :]

    f32 = np.float32
    return dict(
        wT=np.ascontiguousarray(
            np.concatenate([H.T, Gpp.T], axis=0), dtype=f32),  # (L+4, L)
        fT=np.ascontiguousarray(Fp.T, dtype=f32),      # (L, 4)  lhsT for F-pass
        cc=np.ascontiguousarray(CCh, dtype=f32),       # (4*HS, K)
        ss=np.ascontiguousarray(SSh, dtype=f32),       # (4*HS, K)
        rr=np.ascontiguousarray(
            np.concatenate([np.full((2 * HS, K), r[0]),
                            np.full((2 * HS, K), r[1])]), dtype=f32),
        r1=float(r[0]), r2=float(r[1]),
    )


# ---------------------------------------------------------------- program
def build_program(r1: float, r2: float, T: int, nblk: int = 512,
                  loadw: int = 2048):
    """Build + compile the per-core Bass program."""
    K = T // L                  # chunks per signal
    COLS = NSIG * K             # total chunk-columns
    assert T % L == 0 and 128 <= nblk <= 512 and nblk % 128 == 0
    assert loadw % nblk == 0 and loadw % 128 == 0

    nc = bacc.Bacc("TRN2", target_bir_lowering=False, debug=False,
                   num_devices=N_CORES)
    x_d = nc.dram_tensor("x", [NSIG, T], dt.float16, kind="ExternalInput").ap()
    y_d = nc.dram_tensor("y", [NSIG, T], dt.float16, kind="ExternalOutput").ap()
    ident_d = nc.dram_tensor("ident", [128, 128], dt.float32,
                             kind="ExternalInput").ap()
    wT_d = nc.dram_tensor("wT", [L + 4, L], dt.float32,
                          kind="ExternalInput").ap()
    fT_d = nc.dram_tensor("fT", [L, 4], dt.float32, kind="ExternalInput").ap()
    cc_d = nc.dram_tensor("cc", [4 * (NSIG // NPIPE), K], dt.float32,
                          kind="ExternalInput").ap()
    ss_d = nc.dram_tensor("ss", [4 * (NSIG // NPIPE), K], dt.float32,
                          kind="ExternalInput").ap()
    rr_d = nc.dram_tensor("rr", [4 * (NSIG // NPIPE), K], dt.float32,
                          kind="ExternalInput").ap()

    x_flat = x_d.rearrange("a b -> (a b)")
    y_flat = y_d.rearrange("a b -> (a b)")

    def segments(c0, c1):
        """Signal segments of global col range: (sig, k0, k1, off)."""
        segs, c = [], c0
        while c < c1:
            n, k = divmod(c, K)
            k1 = min(K, k + (c1 - c))
            segs.append((n, k, k1, c - c0))
            c += k1 - k
        return segs

    ring = [0]

    def hwdge():
        ring[0] += 1
        return nc.sync if ring[0] % 2 else nc.scalar

    def copy_any(use_act, out_ap, in_ap):
        if use_act:
            nc.scalar.copy(out_ap, in_ap)
        else:
            nc.vector.tensor_copy(out_ap, in_ap)

    HS = NSIG // NPIPE              # signals per pipeline segment
    HCOLS = HS * K
    HROWS = 4 * HS

    with tile.TileContext(nc) as tc, ExitStack() as ctx:
        consts = ctx.enter_context(tc.tile_pool(name="consts", bufs=1))
        scanp = ctx.enter_context(tc.tile_pool(name="scan", bufs=1))
        xtp = ctx.enter_context(tc.tile_pool(name="xt", bufs=1))
        ldp = ctx.enter_context(tc.tile_pool(name="ld", bufs=4))
        fsbp = ctx.enter_context(tc.tile_pool(name="fsb", bufs=2))
        dramp = ctx.enter_context(tc.tile_pool(name="dram", bufs=1, space="DRAM"))
        youtp = ctx.enter_context(tc.tile_pool(name="yout", bufs=3))
        ps_t = ctx.enter_context(tc.tile_pool(name="ps_t", bufs=2, space="PSUM"))
        ps_f = ctx.enter_context(tc.tile_pool(name="ps_f", bufs=1, space="PSUM"))
        ps_y = ctx.enter_context(tc.tile_pool(name="ps_y", bufs=3, space="PSUM"))

        # ---- constants, cast once to fp16 for the PE
        identf = consts.tile([128, 128], dt.float32)
        nc.sync.dma_start(identf[:], ident_d[:])
        identb = consts.tile([128, 128], dt.float16)
        nc.scalar.copy(identb[:], identf[:])
        wT = consts.tile([L + 4, L], dt.float32)
        nc.sync.dma_start(wT[:], wT_d[:])
        wTb = consts.tile([L + 4, L], dt.float16)
        nc.scalar.copy(wTb[:], wT[:])
        fT = consts.tile([L, 4], dt.float32)
        nc.sync.dma_start(fT[:], fT_d[:])
        fTb = consts.tile([L, 4], dt.float16)
        nc.scalar.copy(fTb[:], fT[:])
        cc = consts.tile([HROWS, K], dt.float32)
        nc.sync.dma_start(cc[:], cc_d[:])
        ss = consts.tile([HROWS, K], dt.float32)
        nc.sync.dma_start(ss[:], ss_d[:])

        Xt = xtp.tile([128, COLS], dt.float16)   # rows 0:L = X^T; L:L+4 = states
        rarr = scanp.tile([HROWS, K], dt.float32)
        nc.sync.dma_start(rarr[:], rr_d[:])

        ci = [0]                 # running copy-engine chooser

        def nxt_copy():
            ci[0] += 1
            return (ci[0] % 2) == 0

        tS_tiles = {}
        gb32_tiles = {}

        def seg_range(h):
            hc0, hc1 = h * HCOLS, (h + 1) * HCOLS
            hblocks = []
            c = hc0
            while c < hc1:
                hblocks.append((c, min(c + nblk, hc1)))
                c += nblk
            return hc0, hc1, hblocks

        def phase1(h):
            hc0, hc1, hblocks = seg_range(h)
            r0 = hc0
            while r0 < hc1:
                w = min(loadw, hc1 - r0)
                assert w % 128 == 0, (r0, w)
                t_in = ldp.tile([128, (loadw // 128) * L], dt.float16,
                                tag="ld")
                nq = w // 128
                view = x_flat[r0 * L:(r0 + w) * L].rearrange(
                    "(q p t) -> p q t", p=128, t=L)
                dst = t_in[:, 0:nq * L].rearrange("p (q t) -> p q t", t=L)
                hwdge().dma_start(dst, view)
                pst, pst_base = None, 0
                for q in range(nq):
                    if pst is None:
                        pst = ps_t.tile([128, 512], dt.float16, tag="pst")
                        pst_base = q * 128
                    nc.tensor.transpose(
                        pst[0:L, q * 128 - pst_base:q * 128 - pst_base + 128],
                        t_in[:, q * L:(q + 1) * L], identb[:])
                    if q * 128 + 128 - pst_base == 512 or q == nq - 1:
                        wgrp = q * 128 + 128 - pst_base
                        copy_any(nxt_copy(),
                                 Xt[0:L, r0 + pst_base:r0 + pst_base + wgrp],
                                 pst[0:L, 0:wgrp])
                        pst = None
                r0 += w

            # F-pass: psum pairs -> fsb staging -> DRAM bounce (4, HCOLS)
            gb32 = dramp.tile([4, HCOLS], dt.float32, tag=f"gb32_{h}")
            gb32_tiles[h] = gb32
            FW = 4 * nblk
            fsb = None
            for gi in range(0, len(hblocks), 2):
                gblocks = hblocks[gi:gi + 2]
                psf = ps_f.tile([4, 2 * nblk], dt.float32, tag="psf")
                for j, (c0, c1) in enumerate(gblocks):
                    nc.tensor.matmul(psf[:, j * nblk:j * nblk + (c1 - c0)],
                                     fTb[:], Xt[0:L, c0:c1],
                                     start=True, stop=True)
                if fsb is None:
                    fsb = fsbp.tile([4, FW], dt.float32, tag="fsb")
                    fsb_base = gblocks[0][0]
                gc0, gc1 = gblocks[0][0], gblocks[-1][1]
                copy_any(nxt_copy(),
                         fsb[:, gc0 - fsb_base:gc1 - fsb_base],
                         psf[:, 0:gc1 - gc0])
                if gc1 - fsb_base == FW or gc1 == hc1:
                    nc.gpsimd.dma_start(gb32[:, fsb_base - hc0:gc1 - hc0],
                                        fsb[:, 0:gc1 - fsb_base])
                    fsb = None


        def phase2(h):
            hc0, hc1, hblocks = seg_range(h)
            gb32 = gb32_tiles[h]
            # g_t[a*HS+n, k] = gb32[a, n*K+k]; flat identity (a*HS+n)*K+k
            gbv = gb32[:].rearrange("a c -> (a c)").rearrange(
                "(r k) -> r k", k=K)
            g_t = scanp.tile([HROWS, K], dt.float32, tag="g_t")
            nc.gpsimd.dma_start(g_t[:], gbv)
            gswap = scanp.tile([HROWS, K], dt.float32, tag="swap")
            for (d0, s0) in ((0, HS), (HS, 0), (2 * HS, 3 * HS),
                             (3 * HS, 2 * HS)):
                nc.gpsimd.dma_start(gswap[d0:d0 + HS, :], gbv[s0:s0 + HS, :])
            gt_tw = scanp.tile([HROWS, K], dt.float32, tag="gt_tw")
            tmp1 = scanp.tile([HROWS, K], dt.float32, tag="scr1")
            tmp2 = scanp.tile([HROWS, K], dt.float32, tag="scr2")
            nc.vector.tensor_mul(tmp1[:], cc[:], g_t[:])
            nc.vector.tensor_mul(tmp2[:], ss[:], gswap[:])
            nc.vector.tensor_add(gt_tw[:], tmp1[:], tmp2[:])
            m_t = scanp.tile([HROWS, K], dt.float32, tag="m_t")
            nc.vector.tensor_tensor_scan(
                m_t[:], rarr[:], gt_tw[:], 0.0,
                mybir.AluOpType.mult, mybir.AluOpType.add)
            mswap = scanp.tile([HROWS, K], dt.float32, tag="swap")
            for (d0, s0) in ((0, HS), (HS, 0), (2 * HS, 3 * HS),
                             (3 * HS, 2 * HS)):
                nc.gpsimd.dma_start(mswap[d0:d0 + HS, :], m_t[s0:s0 + HS, :])
            # tS[:, k+1] = cc*m - ss*mswap ; tS[:, 0] = 0  (fp16 for G-pass)
            tS = scanp.tile([HROWS, K + 1], dt.float16, tag=f"tS{h}")
            tS_tiles[h] = tS
            nc.vector.memset(tS[:, 0:1], 0.0)
            t1b = scanp.tile([HROWS, K], dt.float32, tag="scr1")
            t2b = scanp.tile([HROWS, K], dt.float32, tag="scr2")
            nc.vector.tensor_mul(t1b[:], cc[:], m_t[:])
            nc.vector.tensor_mul(t2b[:], ss[:], mswap[:])
            with nc.allow_low_precision(reason="fp16 G-pass operand"):
                nc.vector.tensor_sub(tS[:, 1:K + 1], t1b[:], t2b[:])


        def phase3(h):
            hc0, hc1, hblocks = seg_range(h)
            tS = tS_tiles[h]
            yout, yo_base = None, 0
            sb_end = hc0
            for bi, (c0, c1) in enumerate(hblocks):
                n = c1 - c0
                assert n == nblk
                if c0 == sb_end:
                    sb_end = min(c0 + 4 * nblk, hc1)
                    for (sn, k0, k1, off) in segments(c0, sb_end):
                        nc.gpsimd.dma_start(
                            Xt[L:L + 4, c0 + off:c0 + off + (k1 - k0)],
                            tS[(sn - h * HS)::HS, k0:k1])
                if yout is None:
                    yout = youtp.tile([128, (loadw // 128) * L], dt.float16,
                                      tag="yout")
                    yo_base = c0
                # fused: y_blk[c, t] = [X | S^T] @ [H^T ; G''^T]
                psy = ps_y.tile([128, 4 * L], dt.float32, tag="psy")
                for q in range(4):
                    qc = c0 + q * 128
                    nc.tensor.matmul(psy[:, q * L:(q + 1) * L],
                                     Xt[0:L + 4, qc:qc + 128], wTb[:],
                                     start=True, stop=True)
                off_y = ((c0 - yo_base) // 128) * L
                with nc.allow_low_precision(reason="fp16 output"):
                    copy_any(nxt_copy(), yout[:, off_y:off_y + 4 * L],
                             psy[:, 0:4 * L])

                if c1 - yo_base >= loadw or c1 == hc1:
                    wq = c1 - yo_base
                    assert wq % 128 == 0
                    view = y_flat[yo_base * L:(yo_base + wq) * L] \
                        .rearrange("(qq p t) -> p qq t", p=128, t=L)
                    srcv = yout[:, 0:(wq // 128) * L].rearrange(
                        "p (qq t) -> p qq t", t=L)
                    hwdge().dma_start(view, srcv)
                    yout = None

        # software pipeline: load segment h+2 while computing segment h
        phase1(0)
        phase1(1)
        for h in range(NPIPE):
            phase2(h)
            if h + 2 < NPIPE:
                phase1(h + 2)
            phase3(h)
    nc.compile()
    return nc


class _Exec:
    """Cached PJRT executable for one built program (8-core shard_map)."""

    def __init__(self, nc):
        import jax
        import jax.numpy as jnp
        from jax.sharding import Mesh, PartitionSpec, NamedSharding
        try:
            from jax.experimental.shard_map import shard_map
        except ImportError:
            from jax import shard_map
        from concourse import bass2jax
        from concourse.bass2jax import _bass_exec_p, partition_id_tensor

        bass2jax.install_neuronx_cc_hook()
        assert nc.dbg_addr is None
        pname = nc.partition_id_tensor.name if nc.partition_id_tensor else None
        in_names, out_names, out_avals, zero_outs = [], [], [], []
        for alloc in nc.m.functions[0].allocations:
            if not isinstance(alloc, mybir.MemoryLocationSet):
                continue
            name = alloc.memorylocations[0].name
            if alloc.kind == "ExternalInput":
                if name != pname:
                    in_names.append(name)
            elif alloc.kind == "ExternalOutput":
                shape = tuple(alloc.tensor_shape)
                dtype = mybir.dt.np(alloc.dtype)
                out_names.append(name)
                out_avals.append(jax.core.ShapedArray(shape, dtype))
                zero_outs.append(np.zeros(shape, dtype))
        n_params = len(in_names)
        all_in = in_names + out_names + ([pname] if pname else [])

        def _body(*args):
            operands = list(args)
            if pname is not None:
                operands.append(partition_id_tensor())
            return tuple(_bass_exec_p.bind(
                *operands,
                out_avals=tuple(out_avals),
                in_names=tuple(all_in),
                out_names=tuple(out_names),
                lowering_input_output_aliases=(),
                sim_require_finite=True,
                sim_require_nnan=True,
                nc=nc,
            ))

        devices = jax.devices()[:N_CORES]
        self.mesh = Mesh(np.asarray(devices), ("core",))
        nin = n_params + len(zero_outs)
        self.fn = jax.jit(shard_map(
            _body, mesh=self.mesh,
            in_specs=(PartitionSpec("core"),) * nin,
            out_specs=(PartitionSpec("core"),) * len(out_names),
            check_rep=False))
        self.sharding = NamedSharding(self.mesh, PartitionSpec("core"))
        self.in_names, self.out_names = in_names, out_names
        self.out_avals, self.zero_outs = out_avals, zero_outs
        self.jax, self.jnp = jax, jnp

    def stage(self, in_maps):
        """device_put concat inputs + zero outs; returns arg list."""
        jax = self.jax
        args = []
        for i, name in enumerate(self.in_names):
            cat = np.concatenate([np.asarray(m[name]) for m in in_maps], 0)
            args.append(jax.device_put(cat, self.sharding))
        for z in self.zero_outs:
            zz = np.zeros((N_CORES * z.shape[0], *z.shape[1:]), z.dtype)
            args.append(jax.device_put(zz, self.sharding))
        return args

    def __call__(self, args):
        outs = self.fn(*args)
        self.jax.block_until_ready(outs)
        return outs


_CACHE: dict = {}
_LAST_RUN: dict = {}


def _get_exec(key, r1, r2, T, nblk, loadw):
    if key not in _CACHE:
        nc = build_program(r1, r2, T, nblk=nblk, loadw=loadw)
        _CACHE[key] = (nc, _Exec(nc))
    return _CACHE[key]


def run_filter(x: np.ndarray, sos: np.ndarray, T: int = T_FULL,
               nblk: int = 512, loadw: int = 2048, time_reps: int = 0):
    """x: (256, T) float32 -> (y (256, T) float32, times list[s])."""
    import time as _time
    K = T // L
    consts = derive_constants(sos, K, HS=NSIG // NPIPE)
    key = (sos.astype(np.float32).tobytes(), T, nblk, loadw)
    nc, ex = _get_exec(key, consts["r1"], consts["r2"], T, nblk, loadw)

    xh = x.astype(np.float16)
    shards = xh.reshape(N_CORES, NSIG, T)
    base = {k: consts[k] for k in ("wT", "fT", "cc", "ss", "rr")}
    base["ident"] = np.eye(128, dtype=np.float32)
    in_maps = [dict(base, x=np.ascontiguousarray(shards[i]))
               for i in range(N_CORES)]
    args = ex.stage(in_maps)
    outs = ex(args)                       # first call compiles + runs
    _LAST_RUN.update(nc=nc, ex=ex, args=args)
    times = []
    for _ in range(time_reps):
        t0 = _time.perf_counter()
        outs2 = ex(args)
        times.append(_time.perf_counter() - t0)
    oi = ex.out_names.index("y")
    y = np.asarray(outs[oi]).astype(np.float32).reshape(N_CORES * NSIG, T)
    return y, times


def kernel(x: np.ndarray, sos: np.ndarray) -> np.ndarray:
    x = np.asarray(x, dtype=np.float32)
    sos = np.asarray(sos, dtype=np.float32)
    y, _ = run_filter(x.reshape(B * C, T_FULL), sos)
    return y.reshape(B, C, T_FULL).astype(np.float32)


# revision 20
# speedup vs baseline: 1.0211x; 1.0211x over previous
"""Butterworth 4th-order lowpass (2 cascaded biquads) on 8 TRN2 NeuronCores.

Algorithm: block state-space decomposition of the IIR cascade.
  - Chunk the time axis into L=128 blocks. Within a chunk, the zero-state
    response is a lower-triangular Toeplitz matmul y_zs = H @ x_chunk (PE).
  - Chunk-boundary states follow s_k = M s_{k-1} + f_k with M = A^L.
    Diagonalize M (2 conjugate eigenpairs); each complex mode is solved by
    a first-order REAL scan (DVE tensor_tensor_scan) via the rotation
    trick  m_k = r m_{k-1} + e^{-i th k} g_k,  shat_k = e^{+i th k} m_k.
  - The state correction is a K=4 matmul y += G'' @ S accumulated in PSUM.

Performance structure (~196 us/core vs 234 us baseline; HW-profiled):
  - fp16 everywhere on the PE: fp32/fp32r matmuls double-pump the array
    (fp32_mode=LOW_HIGH, 2x LDWEIGHTS + 2x MATMUL passes) while fp16
    runs single-pass. fp16 over bf16 because the Toeplitz matmul
    amplifies coefficient rounding ~10x (|x|-scale products cancel to
    |y|-scale outputs); fp16's 10 mantissa bits keep end-to-end error
    ~2e-3 against the 2e-2 gate (bf16 measured 1.3e-2). States |shat|
    <= ~8e3 fit fp16 range. The DVE chunk scan stays fp32.
  - x is pre-cast to fp16 on the HOST and y is returned fp16 (cast back
    to f32 on host): halves both HBM streams to 6.1 MB/core each.
  - y is computed CHUNK-MAJOR directly in PSUM (no output transposes):
      y_blk[c, t] = X[c,:] @ H^T  +  S[:,c]^T @ G''^T
    with the DATA (Xt block / state block) as the matmul stationary and
    the constant (H^T / G''^T) as the moving operand, so one PSUM->SBUF
    copy feeds the store DMA. Input transposes stay on the PE (fp16,
    single-pass): DMA-xbar transposes were tried and are individually
    fast but the compiler serializes them against all SBUF<->SBUF DMA
    traffic (deadlock guard), which cost more than they saved.
  - loads ride the SP HWDGE ring; stores ride the ACT HWDGE ring, so
    the two HBM streams drain in parallel. The f-bounce, swaps and
    state gathers go through gpsimd SWDGE.
  - PSUM->SBUF copies alternate scalar/vector (gpsimd cannot touch PSUM
    and its tensor_copy is ~6x slower anyway - measured).
Sharding: 256 independent signals, 32 per core, no cross-core comm.
"""
import numpy as np
from contextlib import ExitStack

import concourse.bass as bass
import concourse.tile as tile
from concourse import bacc, mybir
from concourse.bass_utils import run_bass_kernel_spmd

dt = mybir.dt

B, C, T_FULL = 32, 8, 96000
NPIPE = 2                      # pipeline segments per core
N_CORES = 8
NSIG = (B * C) // N_CORES      # 32 signals per core
L = 120                        # chunk length (96000/120=800; fused H+G needs L+4<=128)


# ---------------------------------------------------------------- host math
def derive_constants(sos: np.ndarray, K: int, HS: int = 16):
    """Constant matrices for the block SSM, float64. K = chunks per signal."""
    sos = sos.astype(np.float64)
    (b0, b1, b2, a1, a2), (B0, B1, B2, A1, A2) = [
        (s[0] / s[3], s[1] / s[3], s[2] / s[3], s[4] / s[3], s[5] / s[3])
        for s in sos
    ]
    c1, c2 = b1 - b0 * a1, b2 - b0 * a2
    A = np.array([
        [-a1, -a2, 0.0, 0.0],
        [1.0, 0.0, 0.0, 0.0],
        [c1, c2, -A1, -A2],
        [0.0, 0.0, 1.0, 0.0],
    ])
    Bv = np.array([1.0, 0.0, b0, 0.0])
    Cv = np.array([B0 * c1, B0 * c2, B1 - B0 * A1, B2 - B0 * A2])
    D = B0 * b0

    h = np.zeros(L)
    h[0] = D
    s = Bv.copy()
    for t in range(1, L):
        h[t] = Cv @ s
        s = A @ s
    H = np.zeros((L, L))
    for j in range(L):
        H[j:, j] = h[: L - j]

    Fm = np.zeros((4, L))
    Ap = np.eye(4)
    for j in range(L - 1, -1, -1):
        Fm[:, j] = Ap @ Bv
        Ap = A @ Ap
    G = np.zeros((L, 4))
    Ap = np.eye(4)
    for t in range(L):
        G[t, :] = Cv @ Ap
        Ap = A @ Ap

    M = np.linalg.matrix_power(A, L)
    lam, V = np.linalg.eig(M)
    idx = [i for i in range(4) if lam[i].imag > 0]
    assert len(idx) == 2, lam
    lam2, V2 = lam[idx], V[:, idx]
    Vinv2 = np.linalg.inv(V)[idx, :]

    Fmod = Vinv2 @ Fm                      # (2, L) complex
    Fp = np.stack([Fmod[0].real, Fmod[0].imag, Fmod[1].real, Fmod[1].imag])
    GV = G @ V2                            # (L, 2) complex
    Gpp = np.stack([2 * GV[:, 0].real, -2 * GV[:, 0].imag,
                    2 * GV[:, 1].real, -2 * GV[:, 1].imag], axis=1)

    r, th = np.abs(lam2), np.angle(lam2)
    k = np.arange(K)
    CCh = np.zeros((4 * HS, K), dtype=np.float64)
    SSh = np.zeros((4 * HS, K), dtype=np.float64)
    for a in range(4):
        e = a // 2
        CCh[a * HS:(a + 1) * HS, :] = np.cos(th[e] * k)[None, :]
        SSh[a * HS:(a + 1) * HS, :] = (1.0 if a % 2 == 0 else -1.0) * \
            np.sin(th[e] * k)[None, :]

    f32 = np.float32
    return dict(
        wT=np.ascontiguousarray(
            np.concatenate([H.T, Gpp.T], axis=0), dtype=f32),  # (L+4, L)
        fT=np.ascontiguousarray(Fp.T, dtype=f32),      # (L, 4)  lhsT for F-pass
        cc=np.ascontiguousarray(CCh, dtype=f32),       # (4*HS, K)
        ss=np.ascontiguousarray(SSh, dtype=f32),       # (4*HS, K)
        rr=np.ascontiguousarray(
            np.concatenate([np.full((2 * HS, K), r[0]),
                            np.full((2 * HS, K), r[1])]), dtype=f32),
        r1=float(r[0]), r2=float(r[1]),
    )


# ---------------------------------------------------------------- program
def build_program(r1: float, r2: float, T: int, nblk: int = 512,
                  loadw: int = 2048):
    """Build + compile the per-core Bass program."""
    K = T // L                  # chunks per signal
    COLS = NSIG * K             # total chunk-columns
    assert T % L == 0 and 128 <= nblk <= 512 and nblk % 128 == 0
    assert loadw % nblk == 0 and loadw % 128 == 0

    nc = bacc.Bacc("TRN2", target_bir_lowering=False, debug=False,
                   num_devices=N_CORES)
    x_d = nc.dram_tensor("x", [NSIG, T], dt.float16, kind="ExternalInput").ap()
    y_d = nc.dram_tensor("y", [NSIG, T], dt.float16, kind="ExternalOutput").ap()
    ident_d = nc.dram_tensor("ident", [128, 128], dt.float32,
                             kind="ExternalInput").ap()
    wT_d = nc.dram_tensor("wT", [L + 4, L], dt.float32,
                          kind="ExternalInput").ap()
    fT_d = nc.dram_tensor("fT", [L, 4], dt.float32, kind="ExternalInput").ap()
    cc_d = nc.dram_tensor("cc", [4 * (NSIG // NPIPE), K], dt.float32,
                          kind="ExternalInput").ap()
    ss_d = nc.dram_tensor("ss", [4 * (NSIG // NPIPE), K], dt.float32,
                          kind="ExternalInput").ap()
    rr_d = nc.dram_tensor("rr", [4 * (NSIG // NPIPE), K], dt.float32,
                          kind="ExternalInput").ap()

    x_flat = x_d.rearrange("a b -> (a b)")
    y_flat = y_d.rearrange("a b -> (a b)")

    def segments(c0, c1):
        """Signal segments of global col range: (sig, k0, k1, off)."""
        segs, c = [], c0
        while c < c1:
            n, k = divmod(c, K)
            k1 = min(K, k + (c1 - c))
            segs.append((n, k, k1, c - c0))
            c += k1 - k
        return segs

    def copy_any(use_act, out_ap, in_ap):
        if use_act:
            nc.scalar.copy(out_ap, in_ap)
        else:
            nc.vector.tensor_copy(out_ap, in_ap)

    HS = NSIG // NPIPE              # signals per pipeline segment
    HCOLS = HS * K
    HROWS = 4 * HS

    with tile.TileContext(nc) as tc, ExitStack() as ctx:
        consts = ctx.enter_context(tc.tile_pool(name="consts", bufs=1))
        scanp = ctx.enter_context(tc.tile_pool(name="scan", bufs=1))
        xtp = ctx.enter_context(tc.tile_pool(name="xt", bufs=1))
        ldp = ctx.enter_context(tc.tile_pool(name="ld", bufs=3))
        fsbp = ctx.enter_context(tc.tile_pool(name="fsb", bufs=2))
        dramp = ctx.enter_context(tc.tile_pool(name="dram", bufs=1, space="DRAM"))
        youtp = ctx.enter_context(tc.tile_pool(name="yout", bufs=2))
        ps_t = ctx.enter_context(tc.tile_pool(name="ps_t", bufs=2, space="PSUM"))
        ps_f = ctx.enter_context(tc.tile_pool(name="ps_f", bufs=1, space="PSUM"))
        ps_y = ctx.enter_context(tc.tile_pool(name="ps_y", bufs=3, space="PSUM"))

        # ---- constants, cast once to fp16 for the PE
        identf = consts.tile([128, 128], dt.float32)
        nc.sync.dma_start(identf[:], ident_d[:])
        identb = consts.tile([128, 128], dt.float16)
        nc.scalar.copy(identb[:], identf[:])
        wT = consts.tile([L + 4, L], dt.float32)
        nc.sync.dma_start(wT[:], wT_d[:])
        wTb = consts.tile([L + 4, L], dt.float16)
        nc.scalar.copy(wTb[:], wT[:])
        fT = consts.tile([L, 4], dt.float32)
        nc.sync.dma_start(fT[:], fT_d[:])
        fTb = consts.tile([L, 4], dt.float16)
        nc.scalar.copy(fTb[:], fT[:])
        cc = consts.tile([HROWS, K], dt.float32)
        nc.sync.dma_start(cc[:], cc_d[:])
        ss = consts.tile([HROWS, K], dt.float32)
        nc.sync.dma_start(ss[:], ss_d[:])

        Xt = xtp.tile([128, COLS], dt.float16)   # rows 0:L = X^T; L:L+4 = states
        rarr = scanp.tile([HROWS, K], dt.float32)
        nc.sync.dma_start(rarr[:], rr_d[:])

        ci = [0]                 # running copy-engine chooser

        def nxt_copy():
            ci[0] += 1
            return (ci[0] % 2) == 0

        tS_tiles = {}
        gb32_tiles = {}

        def seg_range(h):
            hc0, hc1 = h * HCOLS, (h + 1) * HCOLS
            hblocks = []
            c = hc0
            while c < hc1:
                hblocks.append((c, min(c + nblk, hc1)))
                c += nblk
            return hc0, hc1, hblocks

        def phase1(h):
            hc0, hc1, hblocks = seg_range(h)
            r0 = hc0
            while r0 < hc1:
                w = min(loadw, hc1 - r0)
                assert w % 128 == 0, (r0, w)
                t_in = ldp.tile([128, (loadw // 128) * L], dt.float16,
                                tag="ld")
                nq = w // 128
                view = x_flat[r0 * L:(r0 + w) * L].rearrange(
                    "(q p t) -> p q t", p=128, t=L)
                dst = t_in[:, 0:nq * L].rearrange("p (q t) -> p q t", t=L)
                nc.sync.dma_start(dst, view)
                pst, pst_base = None, 0
                for q in range(nq):
                    if pst is None:
                        pst = ps_t.tile([128, 512], dt.float16, tag="pst")
                        pst_base = q * 128
                    nc.tensor.transpose(
                        pst[0:L, q * 128 - pst_base:q * 128 - pst_base + 128],
                        t_in[:, q * L:(q + 1) * L], identb[:])
                    if q * 128 + 128 - pst_base == 512 or q == nq - 1:
                        wgrp = q * 128 + 128 - pst_base
                        copy_any(nxt_copy(),
                                 Xt[0:L, r0 + pst_base:r0 + pst_base + wgrp],
                                 pst[0:L, 0:wgrp])
                        pst = None
                r0 += w

            # F-pass: psum pairs -> fsb staging -> DRAM bounce (4, HCOLS)
            gb32 = dramp.tile([4, HCOLS], dt.float32, tag=f"gb32_{h}")
            gb32_tiles[h] = gb32
            FW = 4 * nblk
            fsb = None
            for gi in range(0, len(hblocks), 2):
                gblocks = hblocks[gi:gi + 2]
                psf = ps_f.tile([4, 2 * nblk], dt.float32, tag="psf")
                for j, (c0, c1) in enumerate(gblocks):
                    nc.tensor.matmul(psf[:, j * nblk:j * nblk + (c1 - c0)],
                                     fTb[:], Xt[0:L, c0:c1],
                                     start=True, stop=True)
                if fsb is None:
                    fsb = fsbp.tile([4, FW], dt.float32, tag="fsb")
                    fsb_base = gblocks[0][0]
                gc0, gc1 = gblocks[0][0], gblocks[-1][1]
                copy_any(nxt_copy(),
                         fsb[:, gc0 - fsb_base:gc1 - fsb_base],
                         psf[:, 0:gc1 - gc0])
                if gc1 - fsb_base == FW or gc1 == hc1:
                    nc.gpsimd.dma_start(gb32[:, fsb_base - hc0:gc1 - hc0],
                                        fsb[:, 0:gc1 - fsb_base])
                    fsb = None


        def phase2(h):
            hc0, hc1, hblocks = seg_range(h)
            gb32 = gb32_tiles[h]
            # g_t[a*HS+n, k] = gb32[a, n*K+k]; flat identity (a*HS+n)*K+k
            gbv = gb32[:].rearrange("a c -> (a c)").rearrange(
                "(r k) -> r k", k=K)
            g_t = scanp.tile([HROWS, K], dt.float32, tag="g_t")
            nc.gpsimd.dma_start(g_t[:], gbv)
            gswap = scanp.tile([HROWS, K], dt.float32, tag="swap")
            for (d0, s0) in ((0, HS), (HS, 0), (2 * HS, 3 * HS),
                             (3 * HS, 2 * HS)):
                nc.gpsimd.dma_start(gswap[d0:d0 + HS, :], gbv[s0:s0 + HS, :])
            gt_tw = scanp.tile([HROWS, K], dt.float32, tag="gt_tw")
            tmp1 = scanp.tile([HROWS, K], dt.float32, tag="scr1")
            tmp2 = scanp.tile([HROWS, K], dt.float32, tag="scr2")
            nc.vector.tensor_mul(tmp1[:], cc[:], g_t[:])
            nc.vector.tensor_mul(tmp2[:], ss[:], gswap[:])
            nc.vector.tensor_add(gt_tw[:], tmp1[:], tmp2[:])
            m_t = scanp.tile([HROWS, K], dt.float32, tag="m_t")
            nc.vector.tensor_tensor_scan(
                m_t[:], rarr[:], gt_tw[:], 0.0,
                mybir.AluOpType.mult, mybir.AluOpType.add)
            mswap = scanp.tile([HROWS, K], dt.float32, tag="swap")
            for (d0, s0) in ((0, HS), (HS, 0), (2 * HS, 3 * HS),
                             (3 * HS, 2 * HS)):
                nc.gpsimd.dma_start(mswap[d0:d0 + HS, :], m_t[s0:s0 + HS, :])
            # tS[:, k+1] = cc*m - ss*mswap ; tS[:, 0] = 0  (fp16 for G-pass)
            tS = scanp.tile([HROWS, K + 1], dt.float16, tag=f"tS{h}")
            tS_tiles[h] = tS
            nc.vector.memset(tS[:, 0:1], 0.0)
            t1b = scanp.tile([HROWS, K], dt.float32, tag="scr1")
            t2b = scanp.tile([HROWS, K], dt.float32, tag="scr2")
            nc.vector.tensor_mul(t1b[:], cc[:], m_t[:])
            nc.vector.tensor_mul(t2b[:], ss[:], mswap[:])
            with nc.allow_low_precision(reason="fp16 G-pass operand"):
                nc.vector.tensor_sub(tS[:, 1:K + 1], t1b[:], t2b[:])


        def phase3(h):
            hc0, hc1, hblocks = seg_range(h)
            tS = tS_tiles[h]
            yout, yo_base = None, 0
            sb_end = hc0
            for bi, (c0, c1) in enumerate(hblocks):
                n = c1 - c0
                assert n == nblk
                if c0 == sb_end:
                    sb_end = min(c0 + 4 * nblk, hc1)
                    for (sn, k0, k1, off) in segments(c0, sb_end):
                        nc.gpsimd.dma_start(
                            Xt[L:L + 4, c0 + off:c0 + off + (k1 - k0)],
                            tS[(sn - h * HS)::HS, k0:k1])
                if yout is None:
                    yout = youtp.tile([128, (loadw // 128) * L], dt.float16,
                                      tag="yout")
                    yo_base = c0
                # fused: y_blk[c, t] = [X | S^T] @ [H^T ; G''^T]
                psy = ps_y.tile([128, 4 * L], dt.float32, tag="psy")
                for q in range(4):
                    qc = c0 + q * 128
                    nc.tensor.matmul(psy[:, q * L:(q + 1) * L],
                                     Xt[0:L + 4, qc:qc + 128], wTb[:],
                                     start=True, stop=True)
                off_y = ((c0 - yo_base) // 128) * L
                with nc.allow_low_precision(reason="fp16 output"):
                    copy_any(nxt_copy(), yout[:, off_y:off_y + 4 * L],
                             psy[:, 0:4 * L])

                if c1 - yo_base >= loadw or c1 == hc1:
                    wq = c1 - yo_base
                    assert wq % 128 == 0
                    view = y_flat[yo_base * L:(yo_base + wq) * L] \
                        .rearrange("(qq p t) -> p qq t", p=128, t=L)
                    srcv = yout[:, 0:(wq // 128) * L].rearrange(
                        "p (qq t) -> p qq t", t=L)
                    nc.scalar.dma_start(view, srcv)
                    yout = None

        # software pipeline: load segment h+2 while computing segment h
        phase1(0)
        phase1(1)
        for h in range(NPIPE):
            phase2(h)
            if h + 2 < NPIPE:
                phase1(h + 2)
            phase3(h)
    nc.compile()
    return nc


class _Exec:
    """Cached PJRT executable for one built program (8-core shard_map)."""

    def __init__(self, nc):
        import jax
        import jax.numpy as jnp
        from jax.sharding import Mesh, PartitionSpec, NamedSharding
        try:
            from jax.experimental.shard_map import shard_map
        except ImportError:
            from jax import shard_map
        from concourse import bass2jax
        from concourse.bass2jax import _bass_exec_p, partition_id_tensor

        bass2jax.install_neuronx_cc_hook()
        assert nc.dbg_addr is None
        pname = nc.partition_id_tensor.name if nc.partition_id_tensor else None
        in_names, out_names, out_avals, zero_outs = [], [], [], []
        for alloc in nc.m.functions[0].allocations:
            if not isinstance(alloc, mybir.MemoryLocationSet):
                continue
            name = alloc.memorylocations[0].name
            if alloc.kind == "ExternalInput":
                if name != pname:
                    in_names.append(name)
            elif alloc.kind == "ExternalOutput":
                shape = tuple(alloc.tensor_shape)
                dtype = mybir.dt.np(alloc.dtype)
                out_names.append(name)
                out_avals.append(jax.core.ShapedArray(shape, dtype))
                zero_outs.append(np.zeros(shape, dtype))
        n_params = len(in_names)
        all_in = in_names + out_names + ([pname] if pname else [])

        def _body(*args):
            operands = list(args)
            if pname is not None:
                operands.append(partition_id_tensor())
            return tuple(_bass_exec_p.bind(
                *operands,
                out_avals=tuple(out_avals),
                in_names=tuple(all_in),
                out_names=tuple(out_names),
                lowering_input_output_aliases=(),
                sim_require_finite=True,
                sim_require_nnan=True,
                nc=nc,
            ))

        devices = jax.devices()[:N_CORES]
        self.mesh = Mesh(np.asarray(devices), ("core",))
        nin = n_params + len(zero_outs)
        self.fn = jax.jit(shard_map(
            _body, mesh=self.mesh,
            in_specs=(PartitionSpec("core"),) * nin,
            out_specs=(PartitionSpec("core"),) * len(out_names),
            check_rep=False))
        self.sharding = NamedSharding(self.mesh, PartitionSpec("core"))
        self.in_names, self.out_names = in_names, out_names
        self.out_avals, self.zero_outs = out_avals, zero_outs
        self.jax, self.jnp = jax, jnp

    def stage(self, in_maps):
        """device_put concat inputs + zero outs; returns arg list."""
        jax = self.jax
        args = []
        for i, name in enumerate(self.in_names):
            cat = np.concatenate([np.asarray(m[name]) for m in in_maps], 0)
            args.append(jax.device_put(cat, self.sharding))
        for z in self.zero_outs:
            zz = np.zeros((N_CORES * z.shape[0], *z.shape[1:]), z.dtype)
            args.append(jax.device_put(zz, self.sharding))
        return args

    def __call__(self, args):
        outs = self.fn(*args)
        self.jax.block_until_ready(outs)
        return outs


_CACHE: dict = {}
_LAST_RUN: dict = {}


def _get_exec(key, r1, r2, T, nblk, loadw):
    if key not in _CACHE:
        nc = build_program(r1, r2, T, nblk=nblk, loadw=loadw)
        _CACHE[key] = (nc, _Exec(nc))
    return _CACHE[key]


def run_filter(x: np.ndarray, sos: np.ndarray, T: int = T_FULL,
               nblk: int = 512, loadw: int = 2048, time_reps: int = 0):
    """x: (256, T) float32 -> (y (256, T) float32, times list[s])."""
    import time as _time
    K = T // L
    consts = derive_constants(sos, K, HS=NSIG // NPIPE)
    key = (sos.astype(np.float32).tobytes(), T, nblk, loadw)
    nc, ex = _get_exec(key, consts["r1"], consts["r2"], T, nblk, loadw)

    xh = x.astype(np.float16)
    shards = xh.reshape(N_CORES, NSIG, T)
    base = {k: consts[k] for k in ("wT", "fT", "cc", "ss", "rr")}
    base["ident"] = np.eye(128, dtype=np.float32)
    in_maps = [dict(base, x=np.ascontiguousarray(shards[i]))
               for i in range(N_CORES)]
    args = ex.stage(in_maps)
    outs = ex(args)                       # first call compiles + runs
    _LAST_RUN.update(nc=nc, ex=ex, args=args)
    times = []
    for _ in range(time_reps):
        t0 = _time.perf_counter()
        outs2 = ex(args)
        times.append(_time.perf_counter() - t0)
    oi = ex.out_names.index("y")
    y = np.asarray(outs[oi]).astype(np.float32).reshape(N_CORES * NSIG, T)
    return y, times


def kernel(x: np.ndarray, sos: np.ndarray) -> np.ndarray:
    x = np.asarray(x, dtype=np.float32)
    sos = np.asarray(sos, dtype=np.float32)
    y, _ = run_filter(x.reshape(B * C, T_FULL), sos)
    return y.reshape(B, C, T_FULL).astype(np.float32)
